# revision 1
# baseline (speedup 1.0000x reference)
"""Trainium2 Bass kernel for nn_ConditionedCategorical (segment_reduce).

Computes, for inputs x_labels [N] (values in [0,16)), y_labels [N] (values in
[0,32)), posterior_estimate [N, 16] fp32:

    numerator[k, y, :] = eps + sum_{n: x_n=k, y_n=y} posterior[n, :]
    out = numerator / numerator.sum(axis=1, keepdims=True)      # [16, 32, 16]

Strategy (data-parallel over 8 NeuronCores, N/8 rows per core):
  - rows are laid out partition-major: row n = p*NT + t maps to SBUF
    (partition p, tile t); each DMA line is contiguous per partition.
  - per 128-row tile: build a [128, 512] fp16 one-hot of the composite label
    (x*32 + y) on the VectorE via a single tensor_scalar(is_equal) against a
    resident iota row, then accumulate numerator[c, s] (+)= post_tile.T @ onehot
    on the TensorE into a single PSUM bank ([16, 512] fp32) across all tiles.
  - AllReduce the [16, 512] partial across the 8 cores, add eps, normalize
    over Y on-device, and DMA the [16, 32, 16] result out (every core emits
    the full output; core 0's copy is returned).
"""

import numpy as np

import concourse.bass as bass
import concourse.tile as tile
from concourse import bacc, mybir
from concourse.bass_utils import run_bass_kernel_spmd

K, Y, C = 16, 32, 16
S = K * Y  # 512 composite buckets
EPS = 1e-8
NCORES = 8
P = 128

f32 = mybir.dt.float32
f16 = mybir.dt.float16
i32 = mybir.dt.int32
i16 = mybir.dt.int16

# last BassKernelResults (for test harness inspection)
last_results = None


def build_nc(nt: int, st: int, repeat: int = 1, single_core: bool = False,
             no_mm: bool = False, fixed_oh: bool = False, psum_banks: int = 1,
             oh_bufs: int = 6, oh_group: int = 1):
    """Build the SPMD program. nt = 128-row tiles per core, st = tiles per
    posterior supertile DMA. repeat re-runs the main loop (PSUM restarts each
    pass, so the result is unchanged) — used for steady-state timing.
    single_core skips the collective (for TimelineSim cost modelling).
    no_mm / fixed_oh are timing-isolation variants (wrong results)."""
    assert nt % st == 0
    assert st % oh_group == 0
    ndev = 1 if single_core else NCORES
    nc = bacc.Bacc("TRN2", target_bir_lowering=False, debug=False, num_devices=ndev)

    xl = nc.declare_dram_parameter("xl", [P, nt], i32, isOutput=False)
    yl = nc.declare_dram_parameter("yl", [P, nt], i32, isOutput=False)
    post = nc.declare_dram_parameter("post", [P, nt * C], f32, isOutput=False)
    out = nc.declare_dram_parameter("out", [K, Y, C], f32, isOutput=True)

    with tile.TileContext(nc) as tc:
        with (
            tc.tile_pool(name="setup", bufs=1) as setup,
            tc.tile_pool(name="persist", bufs=1) as persist,
            tc.tile_pool(name="postf32", bufs=3) as postf32_pool,
            tc.tile_pool(name="postf16", bufs=3) as postf16_pool,
            tc.tile_pool(name="oh", bufs=oh_bufs) as oh_pool,
            tc.tile_pool(name="acc", bufs=1, space="PSUM") as acc_pool,
            tc.tile_pool(name="epi", bufs=1) as epi,
            tc.tile_pool(name="dram", bufs=1, space="DRAM") as dram,
        ):
            # --- setup: iota row + composite labels ---
            iota_i = setup.tile([P, S], i16)
            nc.gpsimd.iota(iota_i[:], pattern=[[1, S]], base=0, channel_multiplier=0)
            iota_f = persist.tile([P, S], f16)
            nc.vector.tensor_copy(iota_f[:], iota_i[:])

            xl_sb = setup.tile([P, nt], i32)
            nc.sync.dma_start(xl_sb[:], xl[:])
            yl_sb = setup.tile([P, nt], i32)
            nc.sync.dma_start(yl_sb[:], yl[:])
            xf = setup.tile([P, nt], f32)
            nc.vector.tensor_copy(xf[:], xl_sb[:])
            yf = setup.tile([P, nt], f32)
            nc.vector.tensor_copy(yf[:], yl_sb[:])
            comp = persist.tile([P, nt], f32)
            # comp = x*32 + y  (exact: values < 512)
            nc.vector.scalar_tensor_tensor(
                comp[:], xf[:], float(Y), yf[:],
                mybir.AluOpType.mult, mybir.AluOpType.add,
            )

            # --- main loop: one-hot + matmul-accumulate ---
            accs = [acc_pool.tile([C, S], f32, name=f"acc{b}", tag=f"acc{b}")
                    for b in range(psum_banks)]
            acc = accs[0]
            fixed_oh_t = None
            if fixed_oh:
                fixed_oh_t = persist.tile([P, S], f16)
                nc.vector.memset(fixed_oh_t[:], 0.0)
            n_super = nt // st
            for rep in range(repeat):
                for sti in range(n_super):
                    pf32 = postf32_pool.tile([P, st * C], f32)
                    nc.sync.dma_start(pf32[:], post[:, sti * st * C:(sti + 1) * st * C])
                    pf16 = postf16_pool.tile([P, st * C], f16)
                    nc.scalar.copy(pf16[:], pf32[:])
                    for j in range(st):
                        t = sti * st + j
                        g = j % oh_group
                        if fixed_oh:
                            oh = fixed_oh_t
                        else:
                            if g == 0:
                                ohg = oh_pool.tile([P, oh_group * S], f16,
                                                   name="ohg", tag="ohg")
                            oh = ohg[:, g * S:(g + 1) * S]
                            nc.vector.tensor_scalar(
                                oh, iota_f[:], comp[:, t:t + 1], None,
                                mybir.AluOpType.is_equal,
                            )
                        if not no_mm:
                            nc.tensor.matmul(
                                accs[t % psum_banks][:],
                                lhsT=pf16[:, j * C:(j + 1) * C],
                                rhs=oh if fixed_oh else ohg[:, g * S:(g + 1) * S],
                                start=(t < psum_banks and rep == 0),
                                stop=(t >= nt - psum_banks and rep == repeat - 1),
                            )
            if no_mm:
                for b in range(psum_banks):
                    nc.vector.memset(accs[b][:], 0.0)
            # --- epilogue: allreduce, eps, normalize over Y, emit ---
            accsb = epi.tile([C, S], f32)
            nc.vector.tensor_copy(accsb[:], acc[:])
            for b in range(1, psum_banks):
                nc.vector.tensor_tensor(accsb[:], accsb[:], accs[b][:],
                                        op=mybir.AluOpType.add)
            cc_in = dram.tile([C, S], f32)
            nc.sync.dma_start(cc_in[:], accsb[:])
            num = epi.tile([C, S], f32)
            if single_core:
                nc.sync.dma_start(num[:], cc_in[:])
            else:
                cc_out = nc.dram_tensor("cc_out", [C, S], f32, addr_space="Shared")
                nc.gpsimd.collective_compute(
                    "AllReduce",
                    mybir.AluOpType.add,
                    replica_groups=[list(range(NCORES))],
                    ins=[cc_in[:]],
                    outs=[cc_out[:]],
                )
                nc.sync.dma_start(num[:], cc_out[:])
            nc.vector.tensor_scalar(
                num[:], num[:], EPS, None, mybir.AluOpType.add,
            )
            den = epi.tile([C, K], f32)
            nc.vector.tensor_reduce(
                den[:],
                num[:].rearrange("c (k y) -> c k y", y=Y),
                axis=mybir.AxisListType.X,
                op=mybir.AluOpType.add,
            )
            rec = epi.tile([C, K], f32)
            nc.vector.reciprocal(rec[:], den[:])
            norm = epi.tile([C, S], f32)
            nc.vector.tensor_tensor(
                norm[:].rearrange("c (k y) -> c k y", y=Y),
                num[:].rearrange("c (k y) -> c k y", y=Y),
                rec[:].unsqueeze(2).broadcast_to((C, K, Y)),
                op=mybir.AluOpType.mult,
            )
            # out[k, y, c] = norm[c, k*Y + y]
            nc.sync.dma_start(
                out[:].rearrange("k y c -> c k y"),
                norm[:].rearrange("c (k y) -> c k y", y=Y),
            )

    nc.compile()
    return nc


_nc_cache = {}


def _get_nc(nt: int, st: int):
    key = (nt, st)
    if key not in _nc_cache:
        _nc_cache[key] = build_nc(nt, st)
    return _nc_cache[key]


def kernel(x_labels, y_labels, posterior_estimate, _trace=False, _tmpdir=None):
    global last_results
    x_labels = np.asarray(x_labels)
    y_labels = np.asarray(y_labels)
    posterior_estimate = np.ascontiguousarray(
        np.asarray(posterior_estimate, dtype=np.float32)
    )
    n = x_labels.shape[0]
    assert n % (NCORES * P) == 0
    nloc = n // NCORES
    nt = nloc // P
    st = 64 if nt % 64 == 0 else (8 if nt % 8 == 0 else 1)

    nc = _get_nc(nt, st)

    xi = np.ascontiguousarray(x_labels.astype(np.int32))
    yi = np.ascontiguousarray(y_labels.astype(np.int32))

    in_maps = []
    for i in range(NCORES):
        sl = slice(i * nloc, (i + 1) * nloc)
        in_maps.append({
            "xl": xi[sl].reshape(P, nt),
            "yl": yi[sl].reshape(P, nt),
            "post": posterior_estimate[sl].reshape(P, nt * C),
        })

    kwargs = {}
    if _trace:
        kwargs.update(trace=True, tmpdir=_tmpdir)
    res = run_bass_kernel_spmd(nc, in_maps, list(range(NCORES)), **kwargs)
    last_results = res
    return res.results[0]["out"]



# revision 13
# speedup vs baseline: 1.8274x; 1.8274x over previous
"""Trainium2 Bass kernel for nn_ConditionedCategorical (segment_reduce).

Computes, for inputs x_labels [N] (values in [0,16)), y_labels [N] (values in
[0,32)), posterior_estimate [N, 16] fp32:

    numerator[k, y, :] = eps + sum_{n: x_n=k, y_n=y} posterior[n, :]
    out = numerator / numerator.sum(axis=1, keepdims=True)      # [16, 32, 16]

Strategy (data-parallel over 8 NeuronCores, N/8 rows per core):
  - rows are laid out partition-major: row n = p*NT + t maps to SBUF
    (partition p, tile t); each DMA line is contiguous per partition.
  - tiles are processed in PAIRS via the fp8e4 DoubleRow matmul (2 fp8
    contraction rows per PE column-cycle, 4x the bf16 single-row path).
    Per pair, the rhs is a [128, 2, 512] fp8 view into two adjacent
    [128, 512] one-hot blocks whose nonzero value is 2^-9 (fp8e4 0x01):
      * DVE/Pool pairs: two int16 tensor_scalar(is_equal) writes (the int16
        value 1 has exactly the 0x01 low byte; the rhs AP reads low bytes
        at stride 2).
      * Act pairs: per tile, abs(iota - comp) then relu(2^-9 - 2^-9*z)
        written directly as stride-2 fp8 (2^-9 at the matching bucket).
    Pair production round-robins over DVE / Pool / Act per `sched` so all
    three engines build one-hots concurrently.
  - posterior tiles are fp8-converted on the Act engine (contiguous
    [128, 32] = DoubleRow lhsT [128, 2, 16] plane-major).
  - the one-hot value is 2^-9, so the epilogue multiplies the accumulated
    [16, 512] numerator by 512 before adding eps.
  - AllReduce the [16, 512] partial across the 8 cores, normalize over Y
    on-device, and DMA the [16, 32, 16] result out.
"""

import numpy as np

import concourse.bass as bass
import concourse.tile as tile
from concourse import bacc, mybir
from concourse.bass_utils import run_bass_kernel_spmd

K, Y, C = 16, 32, 16
S = K * Y  # 512 composite buckets
EPS = 1e-8
NCORES = 8
P = 128
ALPHA = 2.0 ** -9  # value of the one-hot nonzero (fp8e4 0x01)

f32 = mybir.dt.float32
f16 = mybir.dt.float16
f8 = mybir.dt.float8e4
i32 = mybir.dt.int32
i16 = mybir.dt.int16

# last BassKernelResults (for test harness inspection)
last_results = None


def make_sched(cycle: int, n_pool: int, n_act: int) -> str:
    """Evenly spread P/A roles through a cycle of D's."""
    sched = ["D"] * cycle
    slots = [round(i * cycle / max(1, n_pool + n_act)) % cycle
             for i in range(n_pool + n_act)]
    # dedupe collisions
    used = set()
    fixed = []
    for s in slots:
        while s in used:
            s = (s + 1) % cycle
        used.add(s)
        fixed.append(s)
    for i, s in enumerate(fixed):
        sched[s] = "P" if i < n_pool else "A"
    return "".join(sched)


def build_nc(nt: int, st: int, repeat: int = 1, single_core: bool = False,
             no_mm: bool = False, pk_bufs: int = 16,
             n_pool: int = 5, n_act: int = 3):
    """Build the SPMD program. nt = 128-row tiles per core (must be even),
    st = tiles per posterior supertile DMA. repeat re-runs the main loop
    (PSUM restarts each pass) for steady-state timing. single_core skips the
    collective (for TimelineSim). no_mm drops the matmuls (timing isolation,
    wrong results). Per supertile, the last n_pool+n_act pair slots are
    produced by Pool/Act (emitted first, consumed last, so their latency
    hides); the rest are DVE just-in-time."""
    assert nt % st == 0 and st % 2 == 0 and nt % 2 == 0
    ndev = 1 if single_core else NCORES
    nc = bacc.Bacc("TRN2", target_bir_lowering=False, debug=False, num_devices=ndev)

    xl = nc.declare_dram_parameter("xl", [P, nt], i32, isOutput=False)
    yl = nc.declare_dram_parameter("yl", [P, nt], i32, isOutput=False)
    post = nc.declare_dram_parameter("post", [P, nt * C], f32, isOutput=False)
    out = nc.declare_dram_parameter("out", [K, Y, C], f32, isOutput=True)

    n_half = st // 2
    assert n_pool + n_act <= n_half
    # roles within a supertile: DVE first, then Pool, then Act (last slots)
    roles = "D" * (n_half - n_pool - n_act) + "P" * n_pool + "A" * n_act
    use_act = n_act > 0

    with tile.TileContext(nc) as tc:
        with (
            tc.tile_pool(name="setup", bufs=1) as setup,
            tc.tile_pool(name="persist", bufs=1) as persist,
            tc.tile_pool(name="postf32", bufs=3) as postf32_pool,
            tc.tile_pool(name="postf8", bufs=3) as postf8_pool,
            tc.tile_pool(name="pk", bufs=pk_bufs) as pk_pool,
            tc.tile_pool(name="pkp", bufs=14) as pkp_pool,
            tc.tile_pool(name="pka", bufs=10) as pka_pool,
            tc.tile_pool(name="atmp", bufs=4) as atmp_pool,
            tc.tile_pool(name="acc", bufs=1, space="PSUM") as acc_pool,
            tc.tile_pool(name="epi", bufs=1) as epi,
            tc.tile_pool(name="dram", bufs=1, space="DRAM") as dram,
        ):
            # --- setup: iota rows + composite labels ---
            iota_i = persist.tile([P, S], i16)
            nc.gpsimd.iota(iota_i[:], pattern=[[1, S]], base=0, channel_multiplier=0)
            if use_act:
                iota_f = persist.tile([P, S], f16)
                nc.vector.tensor_copy(iota_f[:], iota_i[:])

            xl_sb = setup.tile([P, nt], i32)
            nc.sync.dma_start(xl_sb[:], xl[:])
            yl_sb = setup.tile([P, nt], i32)
            nc.sync.dma_start(yl_sb[:], yl[:])
            xf = setup.tile([P, nt], f32)
            nc.vector.tensor_copy(xf[:], xl_sb[:])
            yf = setup.tile([P, nt], f32)
            nc.vector.tensor_copy(yf[:], yl_sb[:])
            comp = persist.tile([P, nt], f32)
            # comp = x*32 + y  (exact: values < 512)
            nc.vector.scalar_tensor_tensor(
                comp[:], xf[:], float(Y), yf[:],
                mybir.AluOpType.mult, mybir.AluOpType.add,
            )
            if use_act:
                negcomp = persist.tile([P, nt], f32)
                nc.vector.tensor_scalar(
                    negcomp[:], comp[:], -1.0, None, mybir.AluOpType.mult,
                )
                alpha_bias = persist.tile([P, 1], f32)
                nc.vector.memset(alpha_bias[:], ALPHA)

            def onehot_dve(dst, t):
                nc.vector.tensor_scalar(
                    dst, iota_i[:], comp[:, t:t + 1], None,
                    mybir.AluOpType.is_equal,
                )

            def onehot_pool(dst, t):
                nc.gpsimd.tensor_scalar(
                    dst, iota_i[:], comp[:, t:t + 1], None,
                    mybir.AluOpType.is_equal,
                )

            def onehot_act(dst_f8, t):
                # z = |iota - comp|; oh = relu(2^-9 - 2^-9 * z)
                tmp = atmp_pool.tile([P, S], f16, name="atmp", tag="atmp")
                nc.scalar.activation(
                    tmp[:], iota_f[:], mybir.ActivationFunctionType.Abs,
                    bias=negcomp[:, t:t + 1], scale=1.0,
                )
                nc.scalar.activation(
                    dst_f8, tmp[:], mybir.ActivationFunctionType.Relu,
                    bias=alpha_bias[:, 0:1], scale=-ALPHA,
                )

            # --- main loop: pair one-hots + DoubleRow matmul-accumulate ---
            acc = acc_pool.tile([C, S], f32, name="acc", tag="acc")
            n_super = nt // st
            n_pairs = nt // 2
            def emit_slow(sti):
                """Emit Pool/Act one-hot production for supertile sti (they
                depend only on labels, so they can run a supertile ahead of
                their consumption). Returns {j: pk tile}."""
                pk_of = {}
                for j in range(n_half):
                    pr = sti * n_half + j
                    role = roles[j]
                    if role == "A":
                        pk = pka_pool.tile([P, 2 * S], i16, name="pka", tag="pka")
                        pk_of[j] = pk
                        pk8 = pk[:].bitcast(f8)
                        for h in range(2):
                            onehot_act(
                                pk8.rearrange("p (x b) -> p x b", b=2)[
                                    :, h * S:(h + 1) * S, 0],
                                2 * pr + h,
                            )
                    elif role == "P":
                        pk = pkp_pool.tile([P, 2 * S], i16, name="pkp", tag="pkp")
                        pk_of[j] = pk
                        onehot_pool(pk[:, 0:S], 2 * pr)
                        onehot_pool(pk[:, S:2 * S], 2 * pr + 1)
                return pk_of

            def emit_post(sti):
                pf32 = postf32_pool.tile([P, st * C], f32)
                nc.sync.dma_start(pf32[:], post[:, sti * st * C:(sti + 1) * st * C])
                pf8 = postf8_pool.tile([P, st * C], f8)
                nc.scalar.copy(pf8[:], pf32[:])
                return pf8

            for rep in range(repeat):
                pf8_next = emit_post(0)
                slow_next = emit_slow(0)
                for sti in range(n_super):
                    pf8 = pf8_next
                    pk_of = slow_next
                    if sti + 1 < n_super:
                        pf8_next = emit_post(sti + 1)
                        slow_next = emit_slow(sti + 1)
                    for j in range(n_half):
                        pr = sti * n_half + j
                        t = 2 * pr
                        if j in pk_of:
                            pk = pk_of[j]
                        else:
                            pk = pk_pool.tile([P, 2 * S], i16, name="pk", tag="pk")
                            onehot_dve(pk[:, 0:S], t)
                            onehot_dve(pk[:, S:2 * S], t + 1)
                        if not no_mm:
                            nc.tensor.matmul(
                                acc[:],
                                lhsT=pf8[:, j * 2 * C:(j + 1) * 2 * C].rearrange(
                                    "p (j c) -> p j c", j=2),
                                rhs=pk[:].bitcast(f8).rearrange(
                                    "p (j s b) -> p j s b", j=2, b=2)[:, :, :, 0],
                                start=(pr == 0 and rep == 0),
                                stop=(pr == n_pairs - 1 and rep == repeat - 1),
                                perf_mode=mybir.MatmulPerfMode.DoubleRow,
                            )
            if no_mm:
                nc.vector.memset(acc[:], 0.0)
            # --- epilogue: allreduce, rescale 2^9, eps, normalize over Y ---
            accsb = epi.tile([C, S], f32)
            nc.vector.tensor_copy(accsb[:], acc[:])
            cc_in = dram.tile([C, S], f32)
            nc.sync.dma_start(cc_in[:], accsb[:])
            num = epi.tile([C, S], f32)
            if single_core:
                nc.sync.dma_start(num[:], cc_in[:])
            else:
                cc_out = nc.dram_tensor("cc_out", [C, S], f32, addr_space="Shared")
                nc.gpsimd.collective_compute(
                    "AllReduce",
                    mybir.AluOpType.add,
                    replica_groups=[list(range(NCORES))],
                    ins=[cc_in[:]],
                    outs=[cc_out[:]],
                )
                nc.sync.dma_start(num[:], cc_out[:])
            # num = num/ALPHA + eps  (undo the one-hot scale)
            nc.vector.tensor_scalar(
                num[:], num[:], 1.0 / ALPHA, EPS,
                mybir.AluOpType.mult, mybir.AluOpType.add,
            )
            den = epi.tile([C, K], f32)
            nc.vector.tensor_reduce(
                den[:],
                num[:].rearrange("c (k y) -> c k y", y=Y),
                axis=mybir.AxisListType.X,
                op=mybir.AluOpType.add,
            )
            rec = epi.tile([C, K], f32)
            nc.vector.reciprocal(rec[:], den[:])
            norm = epi.tile([C, S], f32)
            nc.vector.tensor_tensor(
                norm[:].rearrange("c (k y) -> c k y", y=Y),
                num[:].rearrange("c (k y) -> c k y", y=Y),
                rec[:].unsqueeze(2).broadcast_to((C, K, Y)),
                op=mybir.AluOpType.mult,
            )
            # out[k, y, c] = norm[c, k*Y + y]
            nc.sync.dma_start(
                out[:].rearrange("k y c -> c k y"),
                norm[:].rearrange("c (k y) -> c k y", y=Y),
            )

    nc.compile()
    return nc


_nc_cache = {}


def _get_nc(nt: int, st: int):
    key = (nt, st)
    if key not in _nc_cache:
        _nc_cache[key] = build_nc(nt, st)
    return _nc_cache[key]


def kernel(x_labels, y_labels, posterior_estimate, _trace=False, _tmpdir=None):
    global last_results
    x_labels = np.asarray(x_labels)
    y_labels = np.asarray(y_labels)
    posterior_estimate = np.ascontiguousarray(
        np.asarray(posterior_estimate, dtype=np.float32)
    )
    n = x_labels.shape[0]
    assert n % (NCORES * P) == 0
    nloc = n // NCORES
    nt = nloc // P
    st = 64 if nt % 64 == 0 else (8 if nt % 8 == 0 else 2)

    nc = _get_nc(nt, st)

    xi = np.ascontiguousarray(x_labels.astype(np.int32))
    yi = np.ascontiguousarray(y_labels.astype(np.int32))

    in_maps = []
    for i in range(NCORES):
        sl = slice(i * nloc, (i + 1) * nloc)
        in_maps.append({
            "xl": xi[sl].reshape(P, nt),
            "yl": yi[sl].reshape(P, nt),
            "post": posterior_estimate[sl].reshape(P, nt * C),
        })

    kwargs = {}
    if _trace:
        kwargs.update(trace=True, tmpdir=_tmpdir)
    res = run_bass_kernel_spmd(nc, in_maps, list(range(NCORES)), **kwargs)
    last_results = res
    return res.results[0]["out"]
